# revision 40
# baseline (speedup 1.0000x reference)
"""Multi-head cross-attention on 8 Trainium2 NeuronCores (bf16 pipeline).

Sharding: data-parallel over batch (2) x tensor-parallel over heads (4 groups
of 4 heads). Core c handles batch c//4, head-group c%4 (a 256-wide slice of
the QKV projection space). Each core computes a partial output-projection
Y_partial = ctx_c @ Wo_c; a ReduceScatter(add) over each batch's 4 cores
leaves each core with a 512-row shard of the summed output, which the host
concatenates.

On-core dataflow (all matmul operands bf16, fp32 PSUM accumulation):
  - x1/x2 are host-cast to bf16 and reach SBUF d-major via XBAR DMA
    transposes ([512,1024] -> [128, 8, 512] with d = chunk*128 + partition),
    so the tensor engine does no transposes at all.
  - K^T/Q^T = W.T @ x^T come out j-major, V = x @ Wv comes out s-major --
    exactly the operand layouts the attention matmuls need.
  - scores are built k-major (S^T) into [128,1024] fp32 PSUM tiles, exp'd in
    one [128,1024] scalar-engine op straight to bf16 SBUF (no max
    subtraction: scores ~ N(0,1)), and fed into the PV matmul. V carries 64
    ones-columns so the softmax denominator lands in PSUM partitions 64..127
    of the same matmul; reciprocal+multiply on DVE normalizes while evicting
    to bf16 SBUF.
  - K/V/Q projection blocks for later slabs are interleaved into the
    attention unit stream so the tensor engine never idles while the scalar
    engine works through the exps.
  - bq/bk are applied on-device (per-partition bias in j-major layout).
    bv/bo commute through softmax/out-projection exactly (softmax rows sum
    to 1), so the host adds bv @ Wo + bo to the final output.
"""

import numpy as np

B, SEQ, D, H, DH = 2, 2048, 1024, 16, 64
N_CORES = 8
GROUPS = 4            # head-groups per batch (cores per batch)
JG = D // GROUPS      # 256 projection dims per core
HPC = H // GROUPS     # 4 heads per core
P = 128

_cached = {}


def _build_program(seq=SEQ, use_f32r=True, with_collective=True,
                   exp_width=1024):
    import concourse.tile as tile
    from concourse import bacc, mybir

    F32 = mybir.dt.float32
    BF16 = mybir.dt.bfloat16
    EXP = mybir.ActivationFunctionType.Exp

    s_chunks = seq // P          # 16  (128-row chunks)
    sb_chunks = seq // 512       # 4   (512-row slabs)
    sk_chunks = seq // 1024      # 2   (1024-wide attention q blocks)
    d_chunks = D // P            # 8
    j_chunks = JG // P           # 2
    PEND_LAG = 15

    nc = bacc.Bacc("TRN2", target_bir_lowering=False, debug=False,
                   num_devices=N_CORES)

    WALL = 4 * d_chunks * JG          # wk|wv images, then wq|wo images
    x1b = nc.dram_tensor("x1b", [seq, D], BF16, kind="ExternalInput")
    x2b = nc.dram_tensor("x2b", [seq, D], BF16, kind="ExternalInput")
    wall = nc.dram_tensor("wall", [P, WALL], BF16, kind="ExternalInput")
    bqr = nc.dram_tensor("bqr", [P, j_chunks], F32, kind="ExternalInput")
    bkr = nc.dram_tensor("bkr", [P, j_chunks], F32, kind="ExternalInput")
    y_out = nc.dram_tensor("y_out", [seq // GROUPS, D], BF16,
                           kind="ExternalOutput")

    with tile.TileContext(nc) as tc:
        with (
            tc.tile_pool(name="consts", bufs=1) as consts,
            tc.tile_pool(name="wpool", bufs=1) as wpool,
            tc.tile_pool(name="x2t", bufs=1) as x2t_pool,
            tc.tile_pool(name="x1t", bufs=1) as x1t_pool,
            tc.tile_pool(name="acts", bufs=1) as acts,
            tc.tile_pool(name="ctp", bufs=2) as ct_pool,
            tc.tile_pool(name="epool", bufs=17) as epool,
            tc.tile_pool(name="small", bufs=4) as small,
            tc.tile_pool(name="ysb", bufs=4) as ysb,
            tc.tile_pool(name="psum_s", bufs=2, space="PSUM") as psum_s,
            tc.tile_pool(name="psum_u", bufs=2, space="PSUM") as psum_u,
            tc.tile_pool(name="psum_mm", bufs=2, space="PSUM") as psum_mm,
            tc.tile_pool(name="dram", bufs=1, space="DRAM") as dram,
        ):
            # ---- input DMAs on two queues (SP: x transposes; ACT: the
            # packed weight wall + biases).  The DMA dispatch path allows
            # only a few outstanding transfers per queue and each one holds
            # its sequencer until the HWDGE slot frees, so the input stream
            # uses few, large transfers ordered by first consumption.
            # x^T via XBAR transpose: out[p, dc, s] = x[s, dc*128 + p]
            wall_sb = wpool.tile([P, WALL], BF16, tag="wall")
            wk_sb = wall_sb[:, 0 * d_chunks * JG:1 * d_chunks * JG].rearrange(
                "p (o j) -> p o j", j=JG)
            wv_sb = wall_sb[:, 1 * d_chunks * JG:2 * d_chunks * JG].rearrange(
                "p (o j) -> p o j", j=JG)
            wq_sb = wall_sb[:, 2 * d_chunks * JG:3 * d_chunks * JG].rearrange(
                "p (o j) -> p o j", j=JG)
            wo_sb = wall_sb[:, 3 * d_chunks * JG:4 * d_chunks * JG].rearrange(
                "p (o n) -> p o n", n=D)
            bq_sb = consts.tile([P, j_chunks], F32, tag="bq")
            bk_sb = consts.tile([P, j_chunks], F32, tag="bk")
            x2Ta = x2t_pool.tile([P, d_chunks, 1024], BF16, tag="x2Ta")
            x2Tb = x2t_pool.tile([P, d_chunks, 1024], BF16, tag="x2Tb")
            x1Ta = x1t_pool.tile([P, d_chunks, 1024], BF16, tag="x1Ta")
            x1Tb = x1t_pool.tile([P, d_chunks, 1024], BF16, tag="x1Tb")

            def xv2(sb):
                t = x2Ta if sb < 2 else x2Tb
                return t[:, :, (sb % 2) * 512:(sb % 2 + 1) * 512]

            def xv1(sb):
                t = x1Ta if sb < 2 else x1Tb
                return t[:, :, (sb % 2) * 512:(sb % 2 + 1) * 512]

            QW = d_chunks * JG
            nc.sync.dma_start(bk_sb[:], bkr[:])
            nc.sync.dma_start(bq_sb[:], bqr[:])
            nc.sync.dma_start(wall_sb[:, 0:QW], wall[:, 0:QW])
            nc.sync.dma_start_transpose(x2Ta[:, :, 0:512], x2b[0:512, :])
            nc.sync.dma_start_transpose(x2Ta[:, :, 512:1024],
                                        x2b[512:1024, :])
            nc.sync.dma_start(wall_sb[:, QW:3 * QW], wall[:, QW:3 * QW])
            nc.sync.dma_start_transpose(x1Ta[:], x1b[0:1024, :])
            nc.sync.dma_start(wall_sb[:, 3 * QW:WALL], wall[:, 3 * QW:WALL])
            nc.sync.dma_start_transpose(x2Tb[:, :, 0:512],
                                        x2b[1024:1536, :])
            nc.sync.dma_start_transpose(x2Tb[:, :, 512:1024],
                                        x2b[1536:2048, :])
            nc.sync.dma_start_transpose(x1Tb[:], x1b[1024:2048, :])

            kT = acts.tile([P, j_chunks, seq], BF16, tag="kT")
            qT = acts.tile([P, j_chunks, seq], BF16, tag="qT")
            # V'' per head-column-block: cols 0..63 V_h, 64..127 ones
            vpp = acts.tile([P, s_chunks, HPC * P], BF16, tag="vpp")

            # Dependency-free warm-up: generate the V'' ones columns on the
            # PE itself (16 accumulating matmuls of exact powers of two) so
            # the tensor engine's p-state ramp is already warm-latched when
            # the first projection matmul arrives.
            ones_seed = consts.tile([P, P], BF16, tag="oseed")
            nc.vector.memset(ones_seed[:], 1.0 / 2048.0)
            ones_rhs = consts.tile([P, 512], BF16, tag="orhs")
            nc.vector.memset(ones_rhs[:], 1.0)
            zero_rhs = consts.tile([P, 512], BF16, tag="zrhs")
            nc.vector.memset(zero_rhs[:], 0.0)
            ones_sb = consts.tile([P, 512], BF16, tag="ones")
            po = psum_mm.tile([P, 512], F32, tag="mm", name="warm_0")
            N_WARM = 32
            for i in range(N_WARM):
                nc.tensor.matmul(po[:], ones_seed[:],
                                 ones_rhs[:] if i < 16 else zero_rhs[:],
                                 start=(i == 0), stop=(i == N_WARM - 1))
            nc.vector.tensor_copy(ones_sb[:], po[:])

            _bridge_n = [0]

            def emit_bridge(n_mm):
                # dependency-free PE filler to keep the p-state ramp hot
                # across a DMA-bound wait
                _bridge_n[0] += 1
                pb = psum_u.tile([P, 512], F32, tag="u",
                                 name=f"bridge_{_bridge_n[0]}")
                for i in range(n_mm):
                    nc.tensor.matmul(pb[:], ones_seed[:], zero_rhs[:],
                                     start=(i == 0), stop=(i == n_mm - 1))
                nc.vector.tensor_copy(zero_rhs[:], pb[:])
            ones_bf = ones_sb[:, 0:DH]

            def emit_ones_fills():
                for si in range(s_chunks):
                    ones_view = vpp[:, si].rearrange(
                        "p (h q) -> p h q", q=P)[:, :, DH:P]
                    nc.vector.tensor_copy(
                        ones_view,
                        ones_bf[:, None, :].to_broadcast([P, HPC, DH]))

            def emit_kproj(sb, jc, halves=False):
                with nc.named_scope("kproj"):
                    pk = psum_mm.tile([P, 512], F32, tag="mm",
                                      name=f"pk_{sb}_{jc}")
                    col_blocks = ((0, 256), (256, 512)) if halves \
                        else ((0, 512),)
                    for c0, c1 in col_blocks:
                        for dc in range(d_chunks):
                            nc.tensor.matmul(
                                pk[:, c0:c1],
                                wk_sb[:, dc, jc * P:(jc + 1) * P],
                                xv2(sb)[:, dc, c0:c1],
                                start=(dc == 0), stop=(dc == d_chunks - 1))
                    nc.vector.tensor_scalar_add(
                        kT[:, jc, sb * 512:(sb + 1) * 512], pk[:],
                        bk_sb[:, jc:jc + 1])

            def emit_qproj(sb, jc):
                with nc.named_scope("qproj"):
                    pk = psum_mm.tile([P, 512], F32, tag="mm",
                                      name=f"pq_{sb}_{jc}")
                    for dc in range(d_chunks):
                        nc.tensor.matmul(
                            pk[:],
                            wq_sb[:, dc, jc * P:(jc + 1) * P],
                            xv1(sb)[:, dc, :],
                            start=(dc == 0), stop=(dc == d_chunks - 1))
                    nc.vector.tensor_scalar_add(
                        qT[:, jc, sb * 512:(sb + 1) * 512], pk[:],
                        bq_sb[:, jc:jc + 1])

            def emit_vproj(sb, q):
                with nc.named_scope("vproj"):
                    si = sb * 4 + q
                    pv = psum_mm.tile([P, 512], F32, tag="mm",
                                      name=f"pv_{sb}_{q}")
                    for dc in range(d_chunks):
                        nc.tensor.matmul(
                            pv[:, :JG],
                            xv2(sb)[:, dc, q * P:(q + 1) * P],
                            wv_sb[:, dc, :],
                            start=(dc == 0), stop=(dc == d_chunks - 1))
                    vv = vpp[:, si].rearrange("p (h q) -> p h q", q=P)[:, :, 0:DH]
                    nc.vector.tensor_copy(
                        vv, pv[:, :JG].rearrange("p (h q) -> p h q", q=DH))

            ybounce = dram.tile([seq, D], BF16, tag="yin")

            cts = {}
            pus_by = {}

            def emit_oproj_unit(sc, s8, tail=False):
                with nc.named_scope("oproj"):
                    cT = cts[sc]
                    si = sc * 8 + s8
                    yt = ysb.tile([P, D], BF16, tag="y", name=f"yt_{sc}_{s8}")
                    # in the tail the score PSUM banks are idle; borrowing
                    # them for odd units doubles the effective ring depth
                    ps_pair = psum_s.tile([P, 1024], F32, tag="s",
                                          name=f"pys_{sc}_{s8}") \
                        if tail and s8 % 2 == 1 else None
                    for nck in range(2):
                        if ps_pair is not None:
                            py = ps_pair[:, nck * 512:(nck + 1) * 512]
                        else:
                            py = psum_mm.tile([P, 512], F32, tag="mm",
                                              name=f"py_{sc}_{s8}_{nck}")[:]
                        for jc in range(j_chunks):
                            nc.tensor.matmul(
                                py,
                                cT[:, jc, s8 * P:(s8 + 1) * P],
                                wo_sb[:, jc, nck * 512:(nck + 1) * 512],
                                start=(jc == 0), stop=(jc == j_chunks - 1))
                        if tail and nck == 1:
                            nc.scalar.copy(
                                yt[:, nck * 512:(nck + 1) * 512], py)
                        else:
                            nc.vector.tensor_copy(
                                yt[:, nck * 512:(nck + 1) * 512], py)
                    nc.sync.dma_start(ybounce[si * P:(si + 1) * P, :], yt[:])
                    if not with_collective and si < (seq // GROUPS) // P:
                        # the metric path's output shard only needs the
                        # first 512 rows; stream them out as they land
                        nc.sync.dma_start(
                            y_out[si * P:(si + 1) * P, :],
                            ybounce[si * P:(si + 1) * P, :])

            def emit_pv(sc, h, kc, et):
                jc, po = h // 2, (h % 2) * DH
                if kc == 0:
                    pus_by[(sc, h)] = [
                        psum_u.tile([P, 512], F32, tag="u",
                                    name=f"pu_{sc}_{h}_{i}")
                        for i in range(2)]
                pus = pus_by[(sc, h)]
                for half in range(2):
                    fsl = slice(half * 512, (half + 1) * 512)
                    nc.tensor.matmul(
                        pus[half][:],
                        vpp[:, kc, h * P:(h + 1) * P],
                        et[:, fsl],
                        start=(kc == 0), stop=(kc == s_chunks - 1))
                    if kc == s_chunks - 1:
                        # normalize this half right away so the PSUM bank
                        # frees before the next head's first PV arrives
                        cT = cts[sc]
                        rt = small.tile([DH, 512], F32, tag="rt",
                                        name=f"rt_{sc}_{h}_{half}")
                        nc.vector.reciprocal(rt[:], pus[half][DH:P, :])
                        nc.vector.tensor_mul(
                            cT[po:po + DH, jc, fsl],
                            pus[half][0:DH, :], rt[:])
                if kc == s_chunks - 1:
                    del pus_by[(sc, h)]

            pend = []

            def emit_attn_unit(sc, h, kc):
              with nc.named_scope("attn"):
                if (h, kc) == (0, 0):
                    cts[sc] = ct_pool.tile([P, j_chunks, 1024], BF16,
                                           tag="cT", name=f"cT_{sc}")
                jc, po = h // 2, (h % 2) * DH
                ps = psum_s.tile([P, 1024], F32, tag="s",
                                 name=f"ps_{sc}_{h}_{kc}")
                for half in range(2):
                    hsl = slice(sc * 1024 + half * 512,
                                sc * 1024 + (half + 1) * 512)
                    nc.tensor.matmul(
                        ps[:, half * 512:(half + 1) * 512],
                        kT[po:po + DH, jc, kc * P:(kc + 1) * P],
                        qT[po:po + DH, jc, hsl],
                        start=True, stop=True)
                et = epool.tile([P, 1024], BF16, tag="e",
                                name=f"et_{sc}_{h}_{kc}")
                nc.scalar.activation(et[:], ps[:], EXP, scale=0.125)
                pend.append((sc, h, kc, et))
                if len(pend) > PEND_LAG:
                    emit_pv(*pend.pop(0))

            # ---- phase A: q slabs for sc=0 first (x1 transpose lands
            # before the x2 pieces), then k/v slab 0; slabs 1-3 interleave
            # into the attention stream ----
            for sb in range(2):
                for jc in range(j_chunks):
                    emit_kproj(sb, jc)
            for sb in range(2):
                for q in range(4):
                    emit_vproj(sb, q)
            emit_bridge(10)
            for sb in range(2):
                for jc in range(j_chunks):
                    emit_qproj(sb, jc)
            emit_ones_fills()

            # interleave plan: (sc, h, kc) -> list of emit callables to run
            # BEFORE that attention unit, fine-grained so the exp pipeline
            # never starves.  k/v slab sb must land before the units that
            # read k-chunks 4sb..4sb+3; q slabs 2,3 (for sc=1) and the sc=0
            # out-projection land inside later heads.
            from functools import partial
            pre = {}

            def add_pre(key, fn):
                pre.setdefault(key, []).append(fn)

            add_pre((0, 0, 2), partial(emit_kproj, 2, 0))
            add_pre((0, 0, 3), partial(emit_kproj, 2, 1))
            add_pre((0, 0, 9), partial(emit_kproj, 3, 0))
            add_pre((0, 0, 10), partial(emit_kproj, 3, 1))
            add_pre((0, 0, 11), partial(emit_vproj, 2, 0))
            add_pre((0, 0, 12), partial(emit_vproj, 2, 1))
            add_pre((0, 0, 13), partial(emit_vproj, 2, 2))
            add_pre((0, 0, 14), partial(emit_vproj, 2, 3))
            add_pre((0, 0, 15), partial(emit_vproj, 3, 0))
            # extras parked at head boundaries hide the PV-accumulator
            # handover latency there
            add_pre((0, 1, 0), partial(emit_vproj, 3, 1))
            add_pre((0, 1, 1), partial(emit_vproj, 3, 2))
            add_pre((0, 1, 2), partial(emit_vproj, 3, 3))
            add_pre((0, 2, 0), partial(emit_qproj, 2, 0))
            add_pre((0, 2, 1), partial(emit_qproj, 2, 1))
            add_pre((0, 3, 0), partial(emit_qproj, 3, 0))
            add_pre((0, 3, 1), partial(emit_qproj, 3, 1))
            # sc0 out-projection spread across sc1, covering its boundaries
            # cT0's last head normalizes at unit (1, 0, PEND_LAG - 1);
            # the sc0 out-projection may only read it after that
            oslots = [(1, 1, 1), (1, 1, 3), (1, 1, 5), (1, 1, 7),
                      (1, 1, 9), (1, 1, 11), (1, 2, 0), (1, 3, 0)]
            for s8 in range(8):
                add_pre(oslots[s8], partial(emit_oproj_unit, 0, s8))

            # ---- attention units ----
            for sc in range(sk_chunks):
                for h in range(HPC):
                    for kc in range(s_chunks):
                        for fn in pre.get((sc, h, kc), ()):
                            fn()
                        emit_attn_unit(sc, h, kc)
                        if (sc, h) == (sk_chunks - 1, HPC - 1):
                            # drain the PV lag so the tail out-projection
                            # starts right after the last exp
                            with nc.named_scope("attn"):
                                while len(pend) > max(0, s_chunks - 2 - kc):
                                    emit_pv(*pend.pop(0))
            assert not pend

            # ---- tail: sc1 out-projection ----
            for s8 in range(8):
                emit_oproj_unit(1, s8, tail=True)

            # ---- sum partials across the 4 cores of this batch ----
            if with_collective:
                half = seq // 2                 # 1024 rows per collective
                qr = seq // GROUPS // 2         # 256 rows per rank per half
                for ci in range(2):
                    ysc = dram.tile([qr, D], BF16, tag="yout",
                                    name=f"ysc_{ci}")
                    nc.gpsimd.collective_compute(
                        "ReduceScatter",
                        mybir.AluOpType.add,
                        replica_groups=[[0, 1, 2, 3], [4, 5, 6, 7]],
                        ins=[ybounce[ci * half:(ci + 1) * half, :].opt()],
                        outs=[ysc[:].opt()],
                    )
                    nc.sync.dma_start(y_out[ci * qr:(ci + 1) * qr, :], ysc[:])
            # (no-collective y_out rows stream out inside emit_oproj_unit)

    nc.compile()
    return nc


def _get_program(seq=SEQ, use_f32r=True):
    key = (seq, use_f32r)
    if key not in _cached:
        _cached[key] = _build_program(seq, use_f32r)
    return _cached[key]


def _bf16(a):
    import ml_dtypes
    return np.ascontiguousarray(np.asarray(a, np.float32)).astype(
        ml_dtypes.bfloat16)


def _pimage(w):
    # [o*128+p, f] -> [p, o*f]: the SBUF image of a p-major weight tile
    o = w.shape[0] // P
    return w.reshape(o, P, -1).transpose(1, 0, 2).reshape(P, -1)


def make_in_maps(x1, x2, Wq, bq, Wk, bk, Wv, bv, Wo, bo):
    """Per-core input dicts for the SPMD program."""
    in_maps = []
    for c in range(N_CORES):
        b, g = c // GROUPS, c % GROUPS
        js = slice(g * JG, (g + 1) * JG)
        wall = np.concatenate([
            _pimage(np.asarray(Wk, np.float32)[:, js]),
            _pimage(np.asarray(Wv, np.float32)[:, js]),
            _pimage(np.asarray(Wq, np.float32)[:, js]),
            _pimage(np.asarray(Wo, np.float32)[js, :]),
        ], axis=1)
        in_maps.append({
            "x1b": _bf16(x1[b]),
            "x2b": _bf16(x2[b]),
            "wall": _bf16(wall),
            "bqr": np.ascontiguousarray(
                np.asarray(bq, np.float32)[js].reshape(2, P).T),
            "bkr": np.ascontiguousarray(
                np.asarray(bk, np.float32)[js].reshape(2, P).T),
        })
    return in_maps


def assemble(results, Wv_bias_fix):
    """results: list of per-core {'y_out': [seq//GROUPS, D] bf16}.

    y_out rows [0:q) = rank's quarter of input rows [0:seq/2);
    rows [q:2q) = rank's quarter of input rows [seq/2:seq)."""
    seq = results[0]["y_out"].shape[0] * GROUPS
    q = seq // GROUPS // 2
    Y = np.empty((B, seq, D), np.float32)
    for c in range(N_CORES):
        b, rr = c // GROUPS, c % GROUPS
        yo = np.asarray(results[c]["y_out"]).astype(np.float32)
        Y[b, rr * q:(rr + 1) * q, :] = yo[:q]
        Y[b, seq // 2 + rr * q:seq // 2 + rr * q + q, :] = yo[q:]
    Y += Wv_bias_fix
    return Y


def kernel(x1, x2, Wq, bq, Wk, bk, Wv, bv, Wo, bo):
    from concourse.bass_utils import run_bass_kernel_spmd

    Wo = np.asarray(Wo, np.float32)
    bv = np.asarray(bv, np.float32)
    bo = np.asarray(bo, np.float32)

    nc = _get_program(SEQ)
    in_maps = make_in_maps(x1, x2, Wq, bq, Wk, bk, Wv, bv, Wo, bo)
    res = run_bass_kernel_spmd(nc, in_maps, core_ids=list(range(N_CORES)))
    fix = (bv @ Wo + bo).astype(np.float32)
    return assemble(res.results, fix)


# revision 47
# speedup vs baseline: 1.0003x; 1.0003x over previous
"""Multi-head cross-attention on 8 Trainium2 NeuronCores (bf16 pipeline).

Sharding: data-parallel over batch (2) x tensor-parallel over heads (4 groups
of 4 heads). Core c handles batch c//4, head-group c%4 (a 256-wide slice of
the QKV projection space). Each core computes a partial output-projection
Y_partial = ctx_c @ Wo_c; a ReduceScatter(add) over each batch's 4 cores
leaves each core with a 512-row shard of the summed output, which the host
concatenates.

On-core dataflow (all matmul operands bf16, fp32 PSUM accumulation):
  - x1/x2 are host-cast to bf16 and reach SBUF d-major via XBAR DMA
    transposes ([512,1024] -> [128, 8, 512] with d = chunk*128 + partition),
    so the tensor engine does no transposes at all.
  - K^T/Q^T = W.T @ x^T come out j-major, V = x @ Wv comes out s-major --
    exactly the operand layouts the attention matmuls need.
  - scores are built k-major (S^T) into [128,1024] fp32 PSUM tiles, exp'd in
    one [128,1024] scalar-engine op straight to bf16 SBUF (no max
    subtraction: scores ~ N(0,1)), and fed into the PV matmul. V carries 64
    ones-columns so the softmax denominator lands in PSUM partitions 64..127
    of the same matmul; reciprocal+multiply on DVE normalizes while evicting
    to bf16 SBUF.
  - K/V/Q projection blocks for later slabs are interleaved into the
    attention unit stream so the tensor engine never idles while the scalar
    engine works through the exps.
  - bq/bk are applied on-device (per-partition bias in j-major layout).
    bv/bo commute through softmax/out-projection exactly (softmax rows sum
    to 1), so the host adds bv @ Wo + bo to the final output.
"""

import numpy as np

B, SEQ, D, H, DH = 2, 2048, 1024, 16, 64
N_CORES = 8
GROUPS = 4            # head-groups per batch (cores per batch)
JG = D // GROUPS      # 256 projection dims per core
HPC = H // GROUPS     # 4 heads per core
P = 128

_cached = {}


def _build_program(seq=SEQ, use_f32r=True, with_collective=True,
                   exp_width=1024):
    import concourse.tile as tile
    from concourse import bacc, mybir

    F32 = mybir.dt.float32
    BF16 = mybir.dt.bfloat16
    EXP = mybir.ActivationFunctionType.Exp

    s_chunks = seq // P          # 16  (128-row chunks)
    sb_chunks = seq // 512       # 4   (512-row slabs)
    sk_chunks = seq // 1024      # 2   (1024-wide attention q blocks)
    d_chunks = D // P            # 8
    j_chunks = JG // P           # 2
    PEND_LAG = 15

    nc = bacc.Bacc("TRN2", target_bir_lowering=False, debug=False,
                   num_devices=N_CORES)

    WALL = 4 * d_chunks * JG          # wk|wv images, then wq|wo images
    x1b = nc.dram_tensor("x1b", [seq, D], BF16, kind="ExternalInput")
    x2b = nc.dram_tensor("x2b", [seq, D], BF16, kind="ExternalInput")
    wall = nc.dram_tensor("wall", [P, WALL], BF16, kind="ExternalInput")
    bqr = nc.dram_tensor("bqr", [P, j_chunks], F32, kind="ExternalInput")
    bkr = nc.dram_tensor("bkr", [P, j_chunks], F32, kind="ExternalInput")
    y_out = nc.dram_tensor("y_out", [seq // GROUPS, D], BF16,
                           kind="ExternalOutput")

    with tile.TileContext(nc) as tc:
        with (
            tc.tile_pool(name="consts", bufs=1) as consts,
            tc.tile_pool(name="wpool", bufs=1) as wpool,
            tc.tile_pool(name="x2t", bufs=1) as x2t_pool,
            tc.tile_pool(name="x1t", bufs=1) as x1t_pool,
            tc.tile_pool(name="acts", bufs=1) as acts,
            tc.tile_pool(name="ctp", bufs=2) as ct_pool,
            tc.tile_pool(name="epool", bufs=17) as epool,
            tc.tile_pool(name="small", bufs=4) as small,
            tc.tile_pool(name="ysb", bufs=4) as ysb,
            tc.tile_pool(name="psum_s", bufs=2, space="PSUM") as psum_s,
            tc.tile_pool(name="psum_u", bufs=2, space="PSUM") as psum_u,
            tc.tile_pool(name="psum_mm", bufs=2, space="PSUM") as psum_mm,
            tc.tile_pool(name="dram", bufs=1, space="DRAM") as dram,
        ):
            # ---- input DMAs on two queues (SP: x transposes; ACT: the
            # packed weight wall + biases).  The DMA dispatch path allows
            # only a few outstanding transfers per queue and each one holds
            # its sequencer until the HWDGE slot frees, so the input stream
            # uses few, large transfers ordered by first consumption.
            # x^T via XBAR transpose: out[p, dc, s] = x[s, dc*128 + p]
            wall_sb = wpool.tile([P, WALL], BF16, tag="wall")
            wk_sb = wall_sb[:, 0 * d_chunks * JG:1 * d_chunks * JG].rearrange(
                "p (o j) -> p o j", j=JG)
            wv_sb = wall_sb[:, 1 * d_chunks * JG:2 * d_chunks * JG].rearrange(
                "p (o j) -> p o j", j=JG)
            wq_sb = wall_sb[:, 2 * d_chunks * JG:3 * d_chunks * JG].rearrange(
                "p (o j) -> p o j", j=JG)
            wo_sb = wall_sb[:, 3 * d_chunks * JG:4 * d_chunks * JG].rearrange(
                "p (o n) -> p o n", n=D)
            bq_sb = consts.tile([P, j_chunks], F32, tag="bq")
            bk_sb = consts.tile([P, j_chunks], F32, tag="bk")
            x2Ta = x2t_pool.tile([P, d_chunks, 1024], BF16, tag="x2Ta")
            x2Tb = x2t_pool.tile([P, d_chunks, 1024], BF16, tag="x2Tb")
            x1Ta = x1t_pool.tile([P, d_chunks, 1024], BF16, tag="x1Ta")
            x1Tb = x1t_pool.tile([P, d_chunks, 1024], BF16, tag="x1Tb")

            def xv2(sb):
                t = x2Ta if sb < 2 else x2Tb
                return t[:, :, (sb % 2) * 512:(sb % 2 + 1) * 512]

            def xv1(sb):
                t = x1Ta if sb < 2 else x1Tb
                return t[:, :, (sb % 2) * 512:(sb % 2 + 1) * 512]

            QW = d_chunks * JG
            nc.sync.dma_start(bk_sb[:], bkr[:])
            nc.sync.dma_start(bq_sb[:], bqr[:])
            nc.sync.dma_start(wall_sb[:, 0:QW], wall[:, 0:QW])
            nc.sync.dma_start_transpose(x2Ta[:, :, 0:512], x2b[0:512, :])
            nc.sync.dma_start_transpose(x2Ta[:, :, 512:1024],
                                        x2b[512:1024, :])
            nc.sync.dma_start(wall_sb[:, QW:3 * QW], wall[:, QW:3 * QW])
            nc.sync.dma_start_transpose(x1Ta[:], x1b[0:1024, :])
            nc.sync.dma_start(wall_sb[:, 3 * QW:WALL], wall[:, 3 * QW:WALL])
            nc.sync.dma_start_transpose(x2Tb[:, :, 0:512],
                                        x2b[1024:1536, :])
            nc.sync.dma_start_transpose(x2Tb[:, :, 512:1024],
                                        x2b[1536:2048, :])
            nc.sync.dma_start_transpose(x1Tb[:], x1b[1024:2048, :])

            kT = acts.tile([P, j_chunks, seq], BF16, tag="kT")
            qT = acts.tile([P, j_chunks, seq], BF16, tag="qT")
            # V'' per head-column-block: cols 0..63 V_h, 64..127 ones
            vpp = acts.tile([P, s_chunks, HPC * P], BF16, tag="vpp")

            # Dependency-free warm-up: generate the V'' ones columns on the
            # PE itself (16 accumulating matmuls of exact powers of two) so
            # the tensor engine's p-state ramp is already warm-latched when
            # the first projection matmul arrives.
            ones_seed = consts.tile([P, P], BF16, tag="oseed")
            nc.vector.memset(ones_seed[:], 1.0 / 2048.0)
            ones_rhs = consts.tile([P, 512], BF16, tag="orhs")
            nc.vector.memset(ones_rhs[:], 1.0)
            zero_rhs = consts.tile([P, 512], BF16, tag="zrhs")
            nc.vector.memset(zero_rhs[:], 0.0)
            ones_sb = consts.tile([P, 512], BF16, tag="ones")
            po = psum_mm.tile([P, 512], F32, tag="mm", name="warm_0")
            N_WARM = 32
            for i in range(N_WARM):
                nc.tensor.matmul(po[:], ones_seed[:],
                                 ones_rhs[:] if i < 16 else zero_rhs[:],
                                 start=(i == 0), stop=(i == N_WARM - 1))
            nc.vector.tensor_copy(ones_sb[:], po[:])

            _bridge_n = [0]

            def emit_bridge(n_mm):
                # dependency-free PE filler to keep the p-state ramp hot
                # across a DMA-bound wait
                _bridge_n[0] += 1
                pb = psum_u.tile([P, 512], F32, tag="u",
                                 name=f"bridge_{_bridge_n[0]}")
                for i in range(n_mm):
                    nc.tensor.matmul(pb[:], ones_seed[:], zero_rhs[:],
                                     start=(i == 0), stop=(i == n_mm - 1))
                nc.vector.tensor_copy(zero_rhs[:], pb[:])
            ones_bf = ones_sb[:, 0:DH]

            def emit_ones_fills():
                for si in range(s_chunks):
                    ones_view = vpp[:, si].rearrange(
                        "p (h q) -> p h q", q=P)[:, :, DH:P]
                    nc.vector.tensor_copy(
                        ones_view,
                        ones_bf[:, None, :].to_broadcast([P, HPC, DH]))

            def emit_kproj(sb, jc, halves=False):
                with nc.named_scope("kproj"):
                    pk = psum_mm.tile([P, 512], F32, tag="mm",
                                      name=f"pk_{sb}_{jc}")
                    col_blocks = ((0, 256), (256, 512)) if halves \
                        else ((0, 512),)
                    for c0, c1 in col_blocks:
                        for dc in range(d_chunks):
                            nc.tensor.matmul(
                                pk[:, c0:c1],
                                wk_sb[:, dc, jc * P:(jc + 1) * P],
                                xv2(sb)[:, dc, c0:c1],
                                start=(dc == 0), stop=(dc == d_chunks - 1))
                    nc.vector.tensor_scalar_add(
                        kT[:, jc, sb * 512:(sb + 1) * 512], pk[:],
                        bk_sb[:, jc:jc + 1])

            def emit_qproj(sb, jc):
                with nc.named_scope("qproj"):
                    pk = psum_mm.tile([P, 512], F32, tag="mm",
                                      name=f"pq_{sb}_{jc}")
                    for dc in range(d_chunks):
                        nc.tensor.matmul(
                            pk[:],
                            wq_sb[:, dc, jc * P:(jc + 1) * P],
                            xv1(sb)[:, dc, :],
                            start=(dc == 0), stop=(dc == d_chunks - 1))
                    nc.vector.tensor_scalar_add(
                        qT[:, jc, sb * 512:(sb + 1) * 512], pk[:],
                        bq_sb[:, jc:jc + 1])

            def emit_vproj(sb, q):
                with nc.named_scope("vproj"):
                    si = sb * 4 + q
                    pv = psum_mm.tile([P, 512], F32, tag="mm",
                                      name=f"pv_{sb}_{q}")
                    for dc in range(d_chunks):
                        nc.tensor.matmul(
                            pv[:, :JG],
                            xv2(sb)[:, dc, q * P:(q + 1) * P],
                            wv_sb[:, dc, :],
                            start=(dc == 0), stop=(dc == d_chunks - 1))
                    vv = vpp[:, si].rearrange("p (h q) -> p h q", q=P)[:, :, 0:DH]
                    nc.vector.tensor_copy(
                        vv, pv[:, :JG].rearrange("p (h q) -> p h q", q=DH))

            ybounce = dram.tile([seq, D], BF16, tag="yin")

            cts = {}
            pus_by = {}

            def emit_oproj_unit(sc, s8, tail=False):
                with nc.named_scope("oproj"):
                    cT = cts[sc]
                    si = sc * 8 + s8
                    yt = ysb.tile([P, D], BF16, tag="y", name=f"yt_{sc}_{s8}")
                    # in the tail the score PSUM banks are idle; borrowing
                    # them for odd units doubles the effective ring depth
                    ps_pair = psum_s.tile([P, 1024], F32, tag="s",
                                          name=f"pys_{sc}_{s8}") \
                        if tail and s8 % 2 == 1 else None
                    for nck in range(2):
                        if ps_pair is not None:
                            py = ps_pair[:, nck * 512:(nck + 1) * 512]
                        else:
                            py = psum_mm.tile([P, 512], F32, tag="mm",
                                              name=f"py_{sc}_{s8}_{nck}")[:]
                        for jc in range(j_chunks):
                            nc.tensor.matmul(
                                py,
                                cT[:, jc, s8 * P:(s8 + 1) * P],
                                wo_sb[:, jc, nck * 512:(nck + 1) * 512],
                                start=(jc == 0), stop=(jc == j_chunks - 1))
                        if tail and nck == 1:
                            nc.scalar.copy(
                                yt[:, nck * 512:(nck + 1) * 512], py)
                        else:
                            nc.vector.tensor_copy(
                                yt[:, nck * 512:(nck + 1) * 512], py)
                    nc.sync.dma_start(ybounce[si * P:(si + 1) * P, :], yt[:])
                    if not with_collective and si < (seq // GROUPS) // P:
                        # the metric path's output shard only needs the
                        # first 512 rows; stream them out as they land
                        nc.sync.dma_start(
                            y_out[si * P:(si + 1) * P, :],
                            ybounce[si * P:(si + 1) * P, :])

            def emit_pv(sc, h, kc, et):
                jc, po = h // 2, (h % 2) * DH
                if kc == 0:
                    pus_by[(sc, h)] = [
                        psum_u.tile([P, 512], F32, tag="u",
                                    name=f"pu_{sc}_{h}_{i}")
                        for i in range(2)]
                pus = pus_by[(sc, h)]
                for half in range(2):
                    fsl = slice(half * 512, (half + 1) * 512)
                    nc.tensor.matmul(
                        pus[half][:],
                        vpp[:, kc, h * P:(h + 1) * P],
                        et[:, fsl],
                        start=(kc == 0), stop=(kc == s_chunks - 1))
                    if kc == s_chunks - 1:
                        # normalize this half right away so the PSUM bank
                        # frees before the next head's first PV arrives
                        cT = cts[sc]
                        rt = small.tile([DH, 512], F32, tag="rt",
                                        name=f"rt_{sc}_{h}_{half}")
                        nc.vector.reciprocal(rt[:], pus[half][DH:P, :])
                        nc.vector.tensor_mul(
                            cT[po:po + DH, jc, fsl],
                            pus[half][0:DH, :], rt[:])
                if kc == s_chunks - 1:
                    del pus_by[(sc, h)]

            pend = []

            def emit_attn_unit(sc, h, kc):
              with nc.named_scope("attn"):
                if (h, kc) == (0, 0):
                    cts[sc] = ct_pool.tile([P, j_chunks, 1024], BF16,
                                           tag="cT", name=f"cT_{sc}")
                jc, po = h // 2, (h % 2) * DH
                ps = psum_s.tile([P, 1024], F32, tag="s",
                                 name=f"ps_{sc}_{h}_{kc}")
                for half in range(2):
                    hsl = slice(sc * 1024 + half * 512,
                                sc * 1024 + (half + 1) * 512)
                    nc.tensor.matmul(
                        ps[:, half * 512:(half + 1) * 512],
                        kT[po:po + DH, jc, kc * P:(kc + 1) * P],
                        qT[po:po + DH, jc, hsl],
                        start=True, stop=True)
                et = epool.tile([P, 1024], BF16, tag="e",
                                name=f"et_{sc}_{h}_{kc}")
                nc.scalar.activation(et[:], ps[:], EXP, scale=0.125)
                pend.append((sc, h, kc, et))
                if len(pend) > PEND_LAG:
                    emit_pv(*pend.pop(0))

            # ---- phase A: q slabs for sc=0 first (x1 transpose lands
            # before the x2 pieces), then k/v slab 0; slabs 1-3 interleave
            # into the attention stream ----
            for sb in range(2):
                for jc in range(j_chunks):
                    emit_kproj(sb, jc)
            for sb in range(2):
                for q in range(4):
                    emit_vproj(sb, q)
            emit_bridge(10)
            for sb in range(2):
                for jc in range(j_chunks):
                    emit_qproj(sb, jc)
            emit_ones_fills()

            # interleave plan: (sc, h, kc) -> list of emit callables to run
            # BEFORE that attention unit, fine-grained so the exp pipeline
            # never starves.  k/v slab sb must land before the units that
            # read k-chunks 4sb..4sb+3; q slabs 2,3 (for sc=1) and the sc=0
            # out-projection land inside later heads.
            from functools import partial
            pre = {}

            def add_pre(key, fn):
                pre.setdefault(key, []).append(fn)

            add_pre((0, 0, 5), partial(emit_kproj, 2, 0))
            add_pre((0, 0, 6), partial(emit_kproj, 2, 1))
            add_pre((0, 0, 9), partial(emit_kproj, 3, 0))
            add_pre((0, 0, 10), partial(emit_kproj, 3, 1))
            add_pre((0, 0, 11), partial(emit_vproj, 2, 0))
            add_pre((0, 0, 12), partial(emit_vproj, 2, 1))
            add_pre((0, 0, 13), partial(emit_vproj, 2, 2))
            add_pre((0, 0, 14), partial(emit_vproj, 2, 3))
            add_pre((0, 0, 15), partial(emit_vproj, 3, 0))
            # extras parked at head boundaries hide the PV-accumulator
            # handover latency there
            add_pre((0, 1, 0), partial(emit_vproj, 3, 1))
            add_pre((0, 1, 1), partial(emit_vproj, 3, 2))
            add_pre((0, 1, 2), partial(emit_vproj, 3, 3))
            add_pre((0, 2, 0), partial(emit_qproj, 2, 0))
            add_pre((0, 2, 1), partial(emit_qproj, 2, 1))
            add_pre((0, 3, 0), partial(emit_qproj, 3, 0))
            add_pre((0, 3, 1), partial(emit_qproj, 3, 1))
            # sc0 out-projection spread across sc1, covering its boundaries
            # cT0's last head normalizes at unit (1, 0, PEND_LAG - 1);
            # the sc0 out-projection may only read it after that
            oslots = [(1, 1, 1), (1, 1, 3), (1, 1, 5), (1, 1, 7),
                      (1, 1, 9), (1, 1, 11), (1, 2, 0), (1, 3, 0)]
            for s8 in range(8):
                add_pre(oslots[s8], partial(emit_oproj_unit, 0, s8))

            # ---- attention units ----
            for sc in range(sk_chunks):
                for h in range(HPC):
                    for kc in range(s_chunks):
                        for fn in pre.get((sc, h, kc), ()):
                            fn()
                        emit_attn_unit(sc, h, kc)
                        if (sc, h) == (sk_chunks - 1, HPC - 1):
                            # drain the PV lag so the tail out-projection
                            # starts right after the last exp
                            with nc.named_scope("attn"):
                                while len(pend) > max(0, s_chunks - 2 - kc):
                                    emit_pv(*pend.pop(0))
            assert not pend

            # ---- tail: sc1 out-projection ----
            for s8 in range(8):
                emit_oproj_unit(1, s8, tail=True)

            # ---- sum partials across the 4 cores of this batch ----
            if with_collective:
                half = seq // 2                 # 1024 rows per collective
                qr = seq // GROUPS // 2         # 256 rows per rank per half
                for ci in range(2):
                    ysc = dram.tile([qr, D], BF16, tag="yout",
                                    name=f"ysc_{ci}")
                    nc.gpsimd.collective_compute(
                        "ReduceScatter",
                        mybir.AluOpType.add,
                        replica_groups=[[0, 1, 2, 3], [4, 5, 6, 7]],
                        ins=[ybounce[ci * half:(ci + 1) * half, :].opt()],
                        outs=[ysc[:].opt()],
                    )
                    nc.sync.dma_start(y_out[ci * qr:(ci + 1) * qr, :], ysc[:])
            # (no-collective y_out rows stream out inside emit_oproj_unit)

    nc.compile()
    return nc


def _get_program(seq=SEQ, use_f32r=True):
    key = (seq, use_f32r)
    if key not in _cached:
        _cached[key] = _build_program(seq, use_f32r)
    return _cached[key]


def _bf16(a):
    import ml_dtypes
    return np.ascontiguousarray(np.asarray(a, np.float32)).astype(
        ml_dtypes.bfloat16)


def _pimage(w):
    # [o*128+p, f] -> [p, o*f]: the SBUF image of a p-major weight tile
    o = w.shape[0] // P
    return w.reshape(o, P, -1).transpose(1, 0, 2).reshape(P, -1)


def make_in_maps(x1, x2, Wq, bq, Wk, bk, Wv, bv, Wo, bo):
    """Per-core input dicts for the SPMD program."""
    in_maps = []
    for c in range(N_CORES):
        b, g = c // GROUPS, c % GROUPS
        js = slice(g * JG, (g + 1) * JG)
        wall = np.concatenate([
            _pimage(np.asarray(Wk, np.float32)[:, js]),
            _pimage(np.asarray(Wv, np.float32)[:, js]),
            _pimage(np.asarray(Wq, np.float32)[:, js]),
            _pimage(np.asarray(Wo, np.float32)[js, :]),
        ], axis=1)
        in_maps.append({
            "x1b": _bf16(x1[b]),
            "x2b": _bf16(x2[b]),
            "wall": _bf16(wall),
            "bqr": np.ascontiguousarray(
                np.asarray(bq, np.float32)[js].reshape(2, P).T),
            "bkr": np.ascontiguousarray(
                np.asarray(bk, np.float32)[js].reshape(2, P).T),
        })
    return in_maps


def assemble(results, Wv_bias_fix):
    """results: list of per-core {'y_out': [seq//GROUPS, D] bf16}.

    y_out rows [0:q) = rank's quarter of input rows [0:seq/2);
    rows [q:2q) = rank's quarter of input rows [seq/2:seq)."""
    seq = results[0]["y_out"].shape[0] * GROUPS
    q = seq // GROUPS // 2
    Y = np.empty((B, seq, D), np.float32)
    for c in range(N_CORES):
        b, rr = c // GROUPS, c % GROUPS
        yo = np.asarray(results[c]["y_out"]).astype(np.float32)
        Y[b, rr * q:(rr + 1) * q, :] = yo[:q]
        Y[b, seq // 2 + rr * q:seq // 2 + rr * q + q, :] = yo[q:]
    Y += Wv_bias_fix
    return Y


def kernel(x1, x2, Wq, bq, Wk, bk, Wv, bv, Wo, bo):
    from concourse.bass_utils import run_bass_kernel_spmd

    Wo = np.asarray(Wo, np.float32)
    bv = np.asarray(bv, np.float32)
    bo = np.asarray(bo, np.float32)

    nc = _get_program(SEQ)
    in_maps = make_in_maps(x1, x2, Wq, bq, Wk, bk, Wv, bv, Wo, bo)
    res = run_bass_kernel_spmd(nc, in_maps, core_ids=list(range(N_CORES)))
    fix = (bv @ Wo + bo).astype(np.float32)
    return assemble(res.results, fix)
